# revision 10
# baseline (speedup 1.0000x reference)
"""Bass/Trainium2 kernel for BidirectionalAttention (RMSNorm + QKV + RoPE +
non-causal attention + out-proj + residual), distributed over 8 NeuronCores.

Sharding: core c handles batch b = c // 4 and head-group hg = c % 4
(4 of the 16 heads, i.e. a 512-wide slice of the qkv/out feature dims).
Each core computes a partial out-projection; the host sums the 4 partials
per batch and adds the residual.

All device matmuls run in float32r (FP22 truncation of fp32 inputs, fp32
PSUM accumulation) which streams at full PE rate for moving dim >= 256.

Layout: everything on device is feature-major ("transposed", [feature,
token]) so no on-device transposes are needed anywhere:
  - qkv^T tiles come straight out of the QKV matmul (with RoPE fused on
    the PSUM->SBUF path),
  - v is produced in natural [token, feature] layout by swapping the
    matmul operand roles (h^T becomes the stationary operand),
  - scores^T = k^T.T @ q^T puts the softmax dim on partitions; the
    denominator is an extra matmul with an all-ones stationary operand,
    whose output is replicated across all 128 partitions so the
    reciprocal broadcasts for free,
  - the RMSNorm sum-of-squares uses the same all-ones trick.
"""

import math
import sys

import numpy as np

for _p in ("/opt/trn_rl_repo", "/root/.axon_site/_ro/trn_rl_repo"):
    if _p not in sys.path:
        sys.path.append(_p)

B = 2
T = 2048
D_MODEL = 2048
N_HEADS = 16
HEAD_DIM = 128
EPS = 1e-6
ROPE_BASE = 10000.0
N_CORES = 8
HG = 4                           # head groups (cores per batch)
HEADS_PER_CORE = N_HEADS // HG   # 4
F = HEADS_PER_CORE * HEAD_DIM    # 512 features per core for each of q/k/v
SCALE = 1.0 / math.sqrt(HEAD_DIM)

KC = D_MODEL // 128  # 16 contraction chunks
TH = T // 2          # phase-A token half

_PROGRAMS: dict = {}
# which matmul sites get float32r operands (bisection/perf knob)
F32R_SITES = frozenset({"ssq", "qk", "v", "sc", "av", "den", "po"})


def _build_program(reps: int = 1):
    """Build + compile the per-core Bass program (SPMD, identical on all cores)."""
    key = (reps, tuple(sorted(F32R_SITES)))
    if key in _PROGRAMS:
        return _PROGRAMS[key]

    import concourse.bacc as bacc
    import concourse.mybir as mybir
    from concourse import tile
    from concourse.bass import ts

    F32 = mybir.dt.float32
    F32R = mybir.dt.float32r
    Act = mybir.ActivationFunctionType

    nc = bacc.Bacc("TRN2", target_bir_lowering=False, debug=False,
                   num_devices=N_CORES)

    xt = nc.dram_tensor("xt", [D_MODEL, T], F32R, kind="ExternalInput")
    wqt = nc.dram_tensor("wqt", [D_MODEL, F], F32R, kind="ExternalInput")
    wkt = nc.dram_tensor("wkt", [D_MODEL, F], F32R, kind="ExternalInput")
    wvt = nc.dram_tensor("wvt", [D_MODEL, F], F32R, kind="ExternalInput")
    wot = nc.dram_tensor("wot", [F, D_MODEL], F32R, kind="ExternalInput")
    cost = nc.dram_tensor("cost", [HEAD_DIM, T], F32, kind="ExternalInput")
    sint = nc.dram_tensor("sint", [HEAD_DIM, T], F32, kind="ExternalInput")
    onesd = nc.dram_tensor("onesd", [128, 128], F32R, kind="ExternalInput")
    po = nc.dram_tensor("po", [D_MODEL, T], F32, kind="ExternalOutput")

    def r(ap, site=""):
        return ap

    def as_f32(ap):
        # read raw f32r bits as plain f32 (non-matmul consumers)
        return ap.bitcast(F32)

    with tile.TileContext(nc) as tc:
        with tc.tile_pool(name="persist", bufs=1) as persist:
            # long-lived SBUF: ~112.5 KB/partition
            cos_t = persist.tile([HEAD_DIM, T], F32, tag="cos")
            sin_t = persist.tile([HEAD_DIM, T], F32, tag="sin")
            nc.sync.dma_start(cos_t[:], cost.ap())
            nc.sync.dma_start(sin_t[:], sint.ap())
            ones_t = persist.tile([128, 128], F32R, tag="ones")
            nc.sync.dma_start(ones_t[:], onesd.ap())
            eps_t = persist.tile([128, 1], F32, tag="eps")
            nc.vector.memset(eps_t[:], EPS)

            qrope = [persist.tile([HEAD_DIM, T], F32R, name=f"qr{h}", tag=f"qr{h}")
                     for h in range(HEADS_PER_CORE)]
            krope = [persist.tile([HEAD_DIM, T], F32R, name=f"kr{h}", tag=f"kr{h}")
                     for h in range(HEADS_PER_CORE)]
            vtiles = [persist.tile([128, F], F32R, name=f"v{u}", tag=f"v{u}")
                      for u in range(KC)]

            for _rep in range(reps):
                # ------------- Phase A: norm + QKV (+RoPE) -------------
                with (
                    tc.tile_pool(name="pa", bufs=1) as pa,
                    tc.tile_pool(name="psa", bufs=8, space="PSUM") as psa,
                ):
                    for half in range(2):
                        tsl = slice(half * TH, (half + 1) * TH)
                        ht = pa.tile([128, KC, TH], F32R, tag="ht")

                        # sum of squares over features (all-ones stationary
                        # operand -> result replicated across partitions)
                        ssq = [psa.tile([128, 512], F32, tag="a",
                                        name=f"ssq{half}_{j}")
                               for j in range(TH // 512)]
                        for kc in range(KC):
                            nc.sync.dma_start(ht[:, kc, :],
                                              xt[ts(kc, 128), tsl])
                            for j in range(TH // 512):
                                sq = pa.tile([128, 512], F32R, tag="sq")
                                nc.scalar.activation(
                                    sq[:], as_f32(ht[:, kc, ts(j, 512)]), Act.Square)
                                nc.tensor.matmul(
                                    ssq[j][:], r(ones_t[:], "ssq"),
                                    r(sq[:], "ssq"),
                                    start=(kc == 0), stop=(kc == KC - 1))

                        # s = 1/sqrt(ssq/D + eps), replicated [128, TH]
                        sstd = pa.tile([128, TH], F32, tag="sstd", name="sstd")
                        for j in range(TH // 512):
                            nc.scalar.activation(
                                sstd[:, ts(j, 512)], ssq[j][:], Act.Sqrt,
                                bias=eps_t[:], scale=1.0 / D_MODEL)
                        nc.vector.reciprocal(sstd[:], sstd[:])

                        # h^T = x^T * s  (in place)
                        for kc in range(KC):
                            nc.vector.tensor_mul(ht[:, kc, :],
                                                 as_f32(ht[:, kc, :]),
                                                 sstd[:])

                        # q^T, k^T with RoPE fused on the PSUM->SBUF path
                        for grp, dest in (("q", qrope), ("k", krope)):
                            wdram = wqt if grp == "q" else wkt
                            acc = [[psa.tile([128, 512], F32, tag="a",
                                             name=f"{grp}{half}_{e}_{j}")
                                    for j in range(TH // 512)]
                                   for e in range(4)]
                            for kc in range(KC):
                                wtile = pa.tile([128, F], F32R, tag="w", bufs=2)
                                nc.sync.dma_start(wtile[:],
                                                  wdram[ts(kc, 128), :])
                                for e in range(4):
                                    for j in range(TH // 512):
                                        nc.tensor.matmul(
                                            acc[e][j][:],
                                            r(wtile[:, ts(e, 128)], "qk"),
                                            r(ht[:, kc, ts(j, 512)], "qk"),
                                            start=(kc == 0),
                                            stop=(kc == KC - 1))
                            for e in range(4):
                                for j in range(TH // 512):
                                    csl = slice(half * TH + j * 512,
                                                half * TH + (j + 1) * 512)
                                    ps = acc[e][j]
                                    tmp = pa.tile([128, 512], F32, tag="rt")
                                    nc.vector.tensor_mul(
                                        tmp[0:64, :], ps[64:128, :],
                                        sin_t[0:64, csl])
                                    nc.vector.tensor_mul(
                                        tmp[64:128, :], ps[0:64, :],
                                        sin_t[64:128, csl])
                                    cosp = pa.tile([128, 512], F32, tag="rc")
                                    nc.vector.tensor_mul(cosp[:], ps[:],
                                                         cos_t[:, csl])
                                    nc.vector.tensor_add(dest[e][:, csl],
                                                         tmp[:], cosp[:])

                        # v in natural [token, feature] layout (h^T stationary)
                        vacc = [psa.tile([128, 512], F32, tag="a",
                                         name=f"va{half}_{u}")
                                for u in range(TH // 128)]
                        for kc in range(KC):
                            wtile = pa.tile([128, F], F32R, tag="w", bufs=2)
                            nc.sync.dma_start(wtile[:], wvt[ts(kc, 128), :])
                            for u in range(TH // 128):
                                nc.tensor.matmul(
                                    vacc[u][:], r(ht[:, kc, ts(u, 128)], "v"),
                                    r(wtile[:], "v"),
                                    start=(kc == 0), stop=(kc == KC - 1))
                        for u in range(TH // 128):
                            nc.vector.tensor_copy(
                                vtiles[half * (TH // 128) + u][:], vacc[u][:])

                # ------------- Phase B: attention + out-proj -------------
                with (
                    tc.tile_pool(name="pb", bufs=1) as pb,
                    tc.tile_pool(name="ps_sc", bufs=2, space="PSUM") as ps_sc,
                    tc.tile_pool(name="ps_av", bufs=4, space="PSUM") as ps_av,
                    tc.tile_pool(name="ps_po", bufs=2, space="PSUM") as ps_po,
                ):
                    wo_tiles = [pb.tile([128, D_MODEL], F32R, name=f"wo{f}", tag=f"wo{f}")
                                for f in range(HEADS_PER_CORE)]
                    for f in range(HEADS_PER_CORE):
                        nc.sync.dma_start(wo_tiles[f][:], wot[ts(f, 128), :])

                    for tt in range(T // 512):
                        oT = [pb.tile([128, 512], F32R, tag=f"oT{h}", bufs=2, name=f"oT{h}")
                              for h in range(HEADS_PER_CORE)]
                        for h in range(HEADS_PER_CORE):
                            out_ps = ps_av.tile([128, 512], F32, tag="av")
                            den_ps = ps_av.tile([128, 512], F32, tag="av")
                            for ut in range(KC):
                                sc = ps_sc.tile([128, 512], F32, tag="sc")
                                nc.tensor.matmul(
                                    sc[:], r(krope[h][:, ts(ut, 128)], "sc"),
                                    r(qrope[h][:, ts(tt, 512)], "sc"))
                                at = pb.tile([128, 512], F32R, tag="at", bufs=3)
                                nc.scalar.activation(at[:], sc[:], Act.Exp,
                                                     scale=SCALE)
                                nc.tensor.matmul(
                                    out_ps[:],
                                    r(vtiles[ut][:, ts(h, 128)], "av"),
                                    r(at[:], "av"),
                                    start=(ut == 0), stop=(ut == KC - 1))
                                nc.tensor.matmul(
                                    den_ps[:], r(ones_t[:], "den"),
                                    r(at[:], "den"),
                                    start=(ut == 0), stop=(ut == KC - 1))
                            rec = pb.tile([128, 512], F32, tag="rec", bufs=2)
                            nc.vector.reciprocal(rec[:], den_ps[:])
                            nc.vector.tensor_mul(oT[h][:], out_ps[:], rec[:])

                        for et in range(D_MODEL // 128):
                            pps = ps_po.tile([128, 512], F32, tag="po")
                            for fc in range(HEADS_PER_CORE):
                                nc.tensor.matmul(
                                    pps[:],
                                    r(wo_tiles[fc][:, ts(et, 128)], "po"),
                                    r(oT[fc][:], "po"),
                                    start=(fc == 0),
                                    stop=(fc == HEADS_PER_CORE - 1))
                            posb = pb.tile([128, 512], F32, tag="posb", bufs=3)
                            nc.vector.tensor_copy(posb[:], pps[:])
                            nc.sync.dma_start(po[ts(et, 128), ts(tt, 512)],
                                              posb[:])

    nc.compile()
    _PROGRAMS[key] = nc
    return nc


def _rope_tables():
    inv_freq = 1.0 / (ROPE_BASE ** (np.arange(0, HEAD_DIM, 2,
                                              dtype=np.float32) / HEAD_DIM))
    t = np.arange(T, dtype=np.float32)
    freqs = np.outer(t, inv_freq)                      # (T, 64)
    emb = np.concatenate([freqs, freqs], axis=-1)      # (T, 128)
    cosT = np.ascontiguousarray(np.cos(emb).T).astype(np.float32)
    sinrotT = np.ascontiguousarray(np.sin(emb).T).astype(np.float32)
    sinrotT[0:64, :] *= -1.0
    return cosT, sinrotT


def _shard_inputs(x, norm_w, w_qkv, w_out):
    cosT, sinrotT = _rope_tables()
    nw = norm_w.astype(np.float32)
    in_maps = []
    for c in range(N_CORES):
        b, hg = divmod(c, HG)
        rs = slice(hg * F, (hg + 1) * F)
        ks = slice(D_MODEL + hg * F, D_MODEL + (hg + 1) * F)
        vs = slice(2 * D_MODEL + hg * F, 2 * D_MODEL + (hg + 1) * F)
        in_maps.append({
            "xt": np.ascontiguousarray(x[b].T),
            "wqt": np.ascontiguousarray((w_qkv[rs] * nw).T),
            "wkt": np.ascontiguousarray((w_qkv[ks] * nw).T),
            "wvt": np.ascontiguousarray((w_qkv[vs] * nw).T),
            "wot": np.ascontiguousarray(w_out[:, rs].T),
            "cost": cosT,
            "sint": sinrotT,
            "onesd": np.ones((128, 128), dtype=np.float32),
        })
    return in_maps


def _gather(results, x):
    y = np.empty((B, T, D_MODEL), dtype=np.float32)
    for b in range(B):
        acc = np.asarray(x[b], dtype=np.float32).copy()
        for hg in range(HG):
            acc += results[b * HG + hg]["po"].T
        y[b] = acc
    return y


def run(x, norm_w, w_qkv, w_out, reps: int = 1):
    from concourse.bass_utils import run_bass_kernel_spmd

    nc = _build_program(reps)
    in_maps = _shard_inputs(x, norm_w, w_qkv, w_out)
    res = run_bass_kernel_spmd(nc, in_maps, core_ids=list(range(N_CORES)))
    return _gather(res.results, x)


def kernel(x, norm_w, w_qkv, w_out):
    return run(np.asarray(x), np.asarray(norm_w), np.asarray(w_qkv),
               np.asarray(w_out))


# revision 13
# speedup vs baseline: 21.6822x; 21.6822x over previous
"""Bass/Trainium2 kernel for BidirectionalAttention (RMSNorm + QKV + RoPE +
non-causal attention + out-proj + residual), distributed over 8 NeuronCores.

Sharding: core c handles batch b = c // 4 and head-group hg = c % 4
(4 of the 16 heads, i.e. a 512-wide slice of the qkv/out feature dims).
Each core computes a partial out-projection; the host sums the 4 partials
per batch and adds the residual.

All device matmuls run in float32r (FP22 truncation of fp32 inputs, fp32
PSUM accumulation) which streams at full PE rate for moving dim >= 256.

Layout: everything on device is feature-major ("transposed", [feature,
token]) so no on-device transposes are needed anywhere:
  - qkv^T tiles come straight out of the QKV matmul (with RoPE fused on
    the PSUM->SBUF path),
  - v is produced in natural [token, feature] layout by swapping the
    matmul operand roles (h^T becomes the stationary operand),
  - scores^T = k^T.T @ q^T puts the softmax dim on partitions; the
    denominator is an extra matmul with an all-ones stationary operand,
    whose output is replicated across all 128 partitions so the
    reciprocal broadcasts for free,
  - the RMSNorm sum-of-squares uses the same all-ones trick.
"""

import math
import sys

import numpy as np

for _p in ("/opt/trn_rl_repo", "/root/.axon_site/_ro/trn_rl_repo"):
    if _p not in sys.path:
        sys.path.append(_p)

B = 2
T = 2048
D_MODEL = 2048
N_HEADS = 16
HEAD_DIM = 128
EPS = 1e-6
ROPE_BASE = 10000.0
N_CORES = 8
HG = 4                           # head groups (cores per batch)
HEADS_PER_CORE = N_HEADS // HG   # 4
F = HEADS_PER_CORE * HEAD_DIM    # 512 features per core for each of q/k/v
SCALE = 1.0 / math.sqrt(HEAD_DIM)

KC = D_MODEL // 128  # 16 contraction chunks
TH = T // 2          # phase-A token half

_PROGRAMS: dict = {}
# which matmul sites get float32r operands (bisection/perf knob)
F32R_SITES = frozenset({"ssq", "qk", "v", "sc", "av", "den", "po"})
USE_F32R = True  # False: build everything in plain fp32 (4 cyc/row PE)


def _build_program(reps: int = 1):
    """Build + compile the per-core Bass program (SPMD, identical on all cores)."""
    key = (reps, USE_F32R, tuple(sorted(F32R_SITES)))
    if key in _PROGRAMS:
        return _PROGRAMS[key]

    import concourse.bacc as bacc
    import concourse.mybir as mybir
    from concourse import tile
    from concourse.bass import ts

    F32 = mybir.dt.float32
    F32R = mybir.dt.float32r if USE_F32R else mybir.dt.float32
    Act = mybir.ActivationFunctionType

    nc = bacc.Bacc("TRN2", target_bir_lowering=False, debug=False,
                   num_devices=N_CORES)

    xt = nc.dram_tensor("xt", [D_MODEL, T], F32R, kind="ExternalInput")
    wqt = nc.dram_tensor("wqt", [D_MODEL, F], F32R, kind="ExternalInput")
    wkt = nc.dram_tensor("wkt", [D_MODEL, F], F32R, kind="ExternalInput")
    wvt = nc.dram_tensor("wvt", [D_MODEL, F], F32R, kind="ExternalInput")
    wot = nc.dram_tensor("wot", [F, D_MODEL], F32R, kind="ExternalInput")
    cost = nc.dram_tensor("cost", [HEAD_DIM, T], F32, kind="ExternalInput")
    sint = nc.dram_tensor("sint", [HEAD_DIM, T], F32, kind="ExternalInput")
    onesd = nc.dram_tensor("onesd", [128, 128], F32R, kind="ExternalInput")
    po = nc.dram_tensor("po", [D_MODEL, T], F32, kind="ExternalOutput")

    def r(ap, site=""):
        return ap

    def as_f32(ap):
        # read raw f32r bits as plain f32 (non-matmul consumers)
        return ap.bitcast(F32)

    with tile.TileContext(nc) as tc:
        with tc.tile_pool(name="persist", bufs=1) as persist:
            # long-lived SBUF: ~112.5 KB/partition
            cos_t = persist.tile([HEAD_DIM, T], F32, tag="cos")
            sin_t = persist.tile([HEAD_DIM, T], F32, tag="sin")
            nc.sync.dma_start(cos_t[:], cost.ap())
            nc.sync.dma_start(sin_t[:], sint.ap())
            ones_t = persist.tile([128, 128], F32R, tag="ones")
            nc.sync.dma_start(ones_t[:], onesd.ap())
            eps_t = persist.tile([128, 1], F32, tag="eps")
            nc.vector.memset(eps_t[:], EPS)

            qrope = [persist.tile([HEAD_DIM, T], F32R, name=f"qr{h}", tag=f"qr{h}")
                     for h in range(HEADS_PER_CORE)]
            krope = [persist.tile([HEAD_DIM, T], F32R, name=f"kr{h}", tag=f"kr{h}")
                     for h in range(HEADS_PER_CORE)]
            vtiles = [persist.tile([128, F], F32R, name=f"v{u}", tag=f"v{u}")
                      for u in range(KC)]

            def emit_body():
                # ------------- Phase A: norm + QKV (+RoPE) -------------
                with (
                    tc.tile_pool(name="pa", bufs=1) as pa,
                    tc.tile_pool(name="psa", bufs=8, space="PSUM") as psa,
                ):
                    for half in range(2):
                        tsl = slice(half * TH, (half + 1) * TH)
                        ht = pa.tile([128, KC, TH], F32R, tag="ht")

                        # sum of squares over features (all-ones stationary
                        # operand -> result replicated across partitions)
                        ssq = [psa.tile([128, 512], F32, tag="a",
                                        name=f"ssq{half}_{j}")
                               for j in range(TH // 512)]
                        for kc in range(KC):
                            nc.sync.dma_start(ht[:, kc, :],
                                              xt[ts(kc, 128), tsl])
                            for j in range(TH // 512):
                                sq = pa.tile([128, 512], F32R, tag="sq")
                                nc.scalar.activation(
                                    sq[:], as_f32(ht[:, kc, ts(j, 512)]), Act.Square)
                                nc.tensor.matmul(
                                    ssq[j][:], r(ones_t[:], "ssq"),
                                    r(sq[:], "ssq"),
                                    start=(kc == 0), stop=(kc == KC - 1))

                        # s = 1/sqrt(ssq/D + eps), replicated [128, TH]
                        sstd = pa.tile([128, TH], F32, tag="sstd", name="sstd")
                        for j in range(TH // 512):
                            nc.scalar.activation(
                                sstd[:, ts(j, 512)], ssq[j][:], Act.Sqrt,
                                bias=eps_t[:], scale=1.0 / D_MODEL)
                        nc.vector.reciprocal(sstd[:], sstd[:])

                        # h^T = x^T * s  (in place)
                        for kc in range(KC):
                            nc.vector.tensor_mul(ht[:, kc, :],
                                                 as_f32(ht[:, kc, :]),
                                                 sstd[:])

                        # q^T, k^T with RoPE fused on the PSUM->SBUF path
                        for grp, dest in (("q", qrope), ("k", krope)):
                            wdram = wqt if grp == "q" else wkt
                            acc = [[psa.tile([128, 512], F32, tag="a",
                                             name=f"{grp}{half}_{e}_{j}")
                                    for j in range(TH // 512)]
                                   for e in range(4)]
                            for kc in range(KC):
                                wtile = pa.tile([128, F], F32R, tag="w", bufs=2)
                                nc.sync.dma_start(wtile[:],
                                                  wdram[ts(kc, 128), :])
                                for e in range(4):
                                    for j in range(TH // 512):
                                        nc.tensor.matmul(
                                            acc[e][j][:],
                                            r(wtile[:, ts(e, 128)], "qk"),
                                            r(ht[:, kc, ts(j, 512)], "qk"),
                                            start=(kc == 0),
                                            stop=(kc == KC - 1))
                            for e in range(4):
                                for j in range(TH // 512):
                                    csl = slice(half * TH + j * 512,
                                                half * TH + (j + 1) * 512)
                                    ps = acc[e][j]
                                    tmp = pa.tile([128, 512], F32, tag="rt")
                                    nc.vector.tensor_mul(
                                        tmp[0:64, :], ps[64:128, :],
                                        sin_t[0:64, csl])
                                    nc.vector.tensor_mul(
                                        tmp[64:128, :], ps[0:64, :],
                                        sin_t[64:128, csl])
                                    cosp = pa.tile([128, 512], F32, tag="rc")
                                    nc.vector.tensor_mul(cosp[:], ps[:],
                                                         cos_t[:, csl])
                                    nc.vector.tensor_add(dest[e][:, csl],
                                                         tmp[:], cosp[:])

                        # v in natural [token, feature] layout (h^T stationary)
                        vacc = [psa.tile([128, 512], F32, tag="a",
                                         name=f"va{half}_{u}")
                                for u in range(TH // 128)]
                        for kc in range(KC):
                            wtile = pa.tile([128, F], F32R, tag="w", bufs=2)
                            nc.sync.dma_start(wtile[:], wvt[ts(kc, 128), :])
                            for u in range(TH // 128):
                                nc.tensor.matmul(
                                    vacc[u][:], r(ht[:, kc, ts(u, 128)], "v"),
                                    r(wtile[:], "v"),
                                    start=(kc == 0), stop=(kc == KC - 1))
                        for u in range(TH // 128):
                            nc.vector.tensor_copy(
                                vtiles[half * (TH // 128) + u][:], vacc[u][:])

                # ------------- Phase B: attention + out-proj -------------
                with (
                    tc.tile_pool(name="pb", bufs=1) as pb,
                    tc.tile_pool(name="ps_sc", bufs=2, space="PSUM") as ps_sc,
                    tc.tile_pool(name="ps_av", bufs=4, space="PSUM") as ps_av,
                    tc.tile_pool(name="ps_po", bufs=2, space="PSUM") as ps_po,
                ):
                    wo_tiles = [pb.tile([128, D_MODEL], F32R, name=f"wo{f}", tag=f"wo{f}")
                                for f in range(HEADS_PER_CORE)]
                    for f in range(HEADS_PER_CORE):
                        nc.sync.dma_start(wo_tiles[f][:], wot[ts(f, 128), :])

                    for tt in range(T // 512):
                        oT = [pb.tile([128, 512], F32R, tag=f"oT{h}", bufs=2, name=f"oT{h}")
                              for h in range(HEADS_PER_CORE)]
                        for h in range(HEADS_PER_CORE):
                            out_ps = ps_av.tile([128, 512], F32, tag="av")
                            den_ps = ps_av.tile([128, 512], F32, tag="av")
                            for ut in range(KC):
                                sc = ps_sc.tile([128, 512], F32, tag="sc")
                                nc.tensor.matmul(
                                    sc[:], r(krope[h][:, ts(ut, 128)], "sc"),
                                    r(qrope[h][:, ts(tt, 512)], "sc"))
                                at = pb.tile([128, 512], F32R, tag="at", bufs=3)
                                nc.scalar.activation(at[:], sc[:], Act.Exp,
                                                     scale=SCALE)
                                nc.tensor.matmul(
                                    out_ps[:],
                                    r(vtiles[ut][:, ts(h, 128)], "av"),
                                    r(at[:], "av"),
                                    start=(ut == 0), stop=(ut == KC - 1))
                                nc.tensor.matmul(
                                    den_ps[:], r(ones_t[:], "den"),
                                    r(at[:], "den"),
                                    start=(ut == 0), stop=(ut == KC - 1))
                            rec = pb.tile([128, 512], F32, tag="rec", bufs=2)
                            nc.vector.reciprocal(rec[:], den_ps[:])
                            nc.vector.tensor_mul(oT[h][:], out_ps[:], rec[:])

                        for et in range(D_MODEL // 128):
                            pps = ps_po.tile([128, 512], F32, tag="po")
                            for fc in range(HEADS_PER_CORE):
                                nc.tensor.matmul(
                                    pps[:],
                                    r(wo_tiles[fc][:, ts(et, 128)], "po"),
                                    r(oT[fc][:], "po"),
                                    start=(fc == 0),
                                    stop=(fc == HEADS_PER_CORE - 1))
                            posb = pb.tile([128, 512], F32, tag="posb", bufs=3)
                            nc.vector.tensor_copy(posb[:], pps[:])
                            nc.sync.dma_start(po[ts(et, 128), ts(tt, 512)],
                                              posb[:])

            if reps == 1:
                emit_body()
            else:
                # hardware loop: NEFF size is independent of `reps`, so
                # wall(reps=R) - wall(reps=1) == (R-1) * body-exec-time
                with tc.For_i(0, reps, 1):
                    emit_body()

    nc.compile()
    _PROGRAMS[key] = nc
    return nc


def _rope_tables():
    inv_freq = 1.0 / (ROPE_BASE ** (np.arange(0, HEAD_DIM, 2,
                                              dtype=np.float32) / HEAD_DIM))
    t = np.arange(T, dtype=np.float32)
    freqs = np.outer(t, inv_freq)                      # (T, 64)
    emb = np.concatenate([freqs, freqs], axis=-1)      # (T, 128)
    cosT = np.ascontiguousarray(np.cos(emb).T).astype(np.float32)
    sinrotT = np.ascontiguousarray(np.sin(emb).T).astype(np.float32)
    sinrotT[0:64, :] *= -1.0
    return cosT, sinrotT


def _shard_inputs(x, norm_w, w_qkv, w_out):
    cosT, sinrotT = _rope_tables()
    nw = norm_w.astype(np.float32)
    in_maps = []
    for c in range(N_CORES):
        b, hg = divmod(c, HG)
        rs = slice(hg * F, (hg + 1) * F)
        ks = slice(D_MODEL + hg * F, D_MODEL + (hg + 1) * F)
        vs = slice(2 * D_MODEL + hg * F, 2 * D_MODEL + (hg + 1) * F)
        in_maps.append({
            "xt": np.ascontiguousarray(x[b].T),
            "wqt": np.ascontiguousarray((w_qkv[rs] * nw).T),
            "wkt": np.ascontiguousarray((w_qkv[ks] * nw).T),
            "wvt": np.ascontiguousarray((w_qkv[vs] * nw).T),
            "wot": np.ascontiguousarray(w_out[:, rs].T),
            "cost": cosT,
            "sint": sinrotT,
            "onesd": np.ones((128, 128), dtype=np.float32),
        })
    return in_maps


def _gather(results, x):
    y = np.empty((B, T, D_MODEL), dtype=np.float32)
    for b in range(B):
        acc = np.asarray(x[b], dtype=np.float32).copy()
        for hg in range(HG):
            acc += results[b * HG + hg]["po"].T
        y[b] = acc
    return y


def run(x, norm_w, w_qkv, w_out, reps: int = 1):
    from concourse.bass_utils import run_bass_kernel_spmd

    nc = _build_program(reps)
    in_maps = _shard_inputs(x, norm_w, w_qkv, w_out)
    res = run_bass_kernel_spmd(nc, in_maps, core_ids=list(range(N_CORES)))
    return _gather(res.results, x)


def kernel(x, norm_w, w_qkv, w_out):
    last_err = None
    for _attempt in range(3):
        try:
            return run(np.asarray(x), np.asarray(norm_w), np.asarray(w_qkv),
                       np.asarray(w_out))
        except Exception as e:  # transient NRT_EXEC_UNIT_UNRECOVERABLE etc.
            last_err = e
    raise last_err


# revision 18
# speedup vs baseline: 142.1417x; 6.5557x over previous
"""Bass/Trainium2 kernel for BidirectionalAttention (RMSNorm + QKV + RoPE +
non-causal attention + out-proj + residual), distributed over 8 NeuronCores.

Sharding: core c handles batch b = c // 4 and head-group hg = c % 4
(4 of the 16 heads, i.e. a 512-wide slice of the qkv/out feature dims).
Each core computes a partial out-projection; the host sums the 4 partials
per batch and adds the residual.

All device matmuls run in float32r (FP22 truncation of fp32 inputs, fp32
PSUM accumulation) which streams at full PE rate for moving dim >= 256.

Layout: everything on device is feature-major ("transposed", [feature,
token]) so no on-device transposes are needed anywhere:
  - qkv^T tiles come straight out of the QKV matmul (with RoPE fused on
    the PSUM->SBUF path),
  - v is produced in natural [token, feature] layout by swapping the
    matmul operand roles (h^T becomes the stationary operand),
  - scores^T = k^T.T @ q^T puts the softmax dim on partitions; the
    denominator is an extra matmul with an all-ones stationary operand,
    whose output is replicated across all 128 partitions so the
    reciprocal broadcasts for free,
  - the RMSNorm sum-of-squares uses the same all-ones trick.
"""

import math
import sys

import numpy as np

for _p in ("/opt/trn_rl_repo", "/root/.axon_site/_ro/trn_rl_repo"):
    if _p not in sys.path:
        sys.path.append(_p)

B = 2
T = 2048
D_MODEL = 2048
N_HEADS = 16
HEAD_DIM = 128
EPS = 1e-6
ROPE_BASE = 10000.0
N_CORES = 8
HG = 4                           # head groups (cores per batch)
HEADS_PER_CORE = N_HEADS // HG   # 4
F = HEADS_PER_CORE * HEAD_DIM    # 512 features per core for each of q/k/v
SCALE = 1.0 / math.sqrt(HEAD_DIM)

KC = D_MODEL // 128  # 16 contraction chunks
TH = T // 2          # phase-A token half

_PROGRAMS: dict = {}
# which matmul sites get float32r operands (bisection/perf knob)
F32R_SITES = frozenset({"ssq", "qk", "v", "sc", "av", "den", "po"})
USE_F32R = True  # False: build everything in plain fp32 (4 cyc/row PE)
SKIP_A = False   # timing bisection: skip phase A
SKIP_B = False   # timing bisection: skip phase B
SKIP_EXP = False  # phase B: replace exp with DVE copy
SKIP_DEN = False  # phase B: skip denominator matmul + normalization
SKIP_NORM = False  # phase A: skip square/ssq/sqrt/scale (use raw x as h)
SKIP_QK = False    # phase A: skip q/k matmuls + rope
SKIP_V = False     # phase A: skip v matmuls


def _build_program(reps: int = 1):
    """Build + compile the per-core Bass program (SPMD, identical on all cores)."""
    key = (reps, USE_F32R, SKIP_A, SKIP_B, SKIP_EXP, SKIP_DEN,
           SKIP_NORM, SKIP_QK, SKIP_V)
    if key in _PROGRAMS:
        return _PROGRAMS[key]

    import concourse.bacc as bacc
    import concourse.mybir as mybir
    from concourse import tile
    from concourse.bass import ts

    F32 = mybir.dt.float32
    F32R = mybir.dt.float32r if USE_F32R else mybir.dt.float32
    Act = mybir.ActivationFunctionType

    nc = bacc.Bacc("TRN2", target_bir_lowering=False, debug=False,
                   num_devices=N_CORES)

    xt = nc.dram_tensor("xt", [D_MODEL, T], F32R, kind="ExternalInput")
    wqt = nc.dram_tensor("wqt", [D_MODEL, F], F32R, kind="ExternalInput")
    wkt = nc.dram_tensor("wkt", [D_MODEL, F], F32R, kind="ExternalInput")
    wvt = nc.dram_tensor("wvt", [D_MODEL, F], F32R, kind="ExternalInput")
    wot = nc.dram_tensor("wot", [F, D_MODEL], F32R, kind="ExternalInput")
    cost = nc.dram_tensor("cost", [HEAD_DIM, T], F32, kind="ExternalInput")
    sint = nc.dram_tensor("sint", [HEAD_DIM, T], F32, kind="ExternalInput")
    onesd = nc.dram_tensor("onesd", [128, 128], F32R, kind="ExternalInput")
    po = nc.dram_tensor("po", [D_MODEL, T], F32, kind="ExternalOutput")

    def r(ap, site=""):
        return ap

    def as_f32(ap):
        # read raw f32r bits as plain f32 (non-matmul consumers)
        return ap.bitcast(F32)

    with tile.TileContext(nc) as tc:
        with tc.tile_pool(name="persist", bufs=1) as persist:
            # long-lived SBUF: ~112.5 KB/partition
            cos_t = persist.tile([HEAD_DIM, T], F32, tag="cos")
            sin_t = persist.tile([HEAD_DIM, T], F32, tag="sin")
            nc.sync.dma_start(cos_t[:], cost.ap())
            nc.sync.dma_start(sin_t[:], sint.ap())
            ones_t = persist.tile([128, 128], F32R, tag="ones")
            nc.sync.dma_start(ones_t[:], onesd.ap())
            eps_t = persist.tile([128, 1], F32, tag="eps")
            nc.vector.memset(eps_t[:], EPS)

            qrope = [persist.tile([HEAD_DIM, T], F32R, name=f"qr{h}", tag=f"qr{h}")
                     for h in range(HEADS_PER_CORE)]
            krope = [persist.tile([HEAD_DIM, T], F32R, name=f"kr{h}", tag=f"kr{h}")
                     for h in range(HEADS_PER_CORE)]
            vtiles = [persist.tile([128, F], F32R, name=f"v{u}", tag=f"v{u}")
                      for u in range(KC)]

            def emit_body():
                # ------------- Phase A: norm + QKV (+RoPE) -------------
                if SKIP_A:
                    pass
                else:
                 with (
                    tc.tile_pool(name="pa", bufs=1) as pa,
                    tc.tile_pool(name="psa", bufs=8, space="PSUM") as psa,
                ):
                    for half in range(2):
                        tsl = slice(half * TH, (half + 1) * TH)
                        ht = pa.tile([128, KC, TH], F32R, tag="ht")

                        # sum of squares over features (all-ones stationary
                        # operand -> result replicated across partitions)
                        ssq = [psa.tile([128, 512], F32, tag="a",
                                        name=f"ssq{half}_{j}")
                               for j in range(TH // 512)]
                        for kc in range(KC):
                            eng = nc.sync if kc % 2 == 0 else nc.gpsimd
                            eng.dma_start(ht[:, kc, :],
                                          xt[ts(kc, 128), tsl])
                            if SKIP_NORM:
                                continue
                            for j in range(TH // 512):
                                sq = pa.tile([128, 512], F32R, tag="sq", bufs=2)
                                nc.scalar.activation(
                                    sq[:], as_f32(ht[:, kc, ts(j, 512)]), Act.Square)
                                nc.tensor.matmul(
                                    ssq[j][:], r(ones_t[:], "ssq"),
                                    r(sq[:], "ssq"),
                                    start=(kc == 0), stop=(kc == KC - 1))

                        # s = 1/sqrt(ssq/D + eps), replicated [128, TH]
                        if SKIP_NORM:
                            for kc in range(KC):
                                nc.vector.tensor_copy(
                                    r(ht[:, kc, :], "qk"),
                                    as_f32(ht[:, kc, :]))
                        sstd = pa.tile([128, TH], F32, tag="sstd", name="sstd")
                        if not SKIP_NORM:
                            for j in range(TH // 512):
                                nc.scalar.activation(
                                    sstd[:, ts(j, 512)], ssq[j][:], Act.Sqrt,
                                    bias=eps_t[:], scale=1.0 / D_MODEL)
                            nc.vector.reciprocal(sstd[:], sstd[:])

                            # h^T = x^T * s  (in place)
                            for kc in range(KC):
                                nc.vector.tensor_mul(ht[:, kc, :],
                                                     as_f32(ht[:, kc, :]),
                                                     sstd[:])

                        # q^T, k^T with RoPE fused on the PSUM->SBUF path
                        for grp, dest in (() if SKIP_QK else
                                          (("q", qrope), ("k", krope))):
                            wdram = wqt if grp == "q" else wkt
                            acc = [[psa.tile([128, 512], F32, tag="a",
                                             name=f"{grp}{half}_{e}_{j}")
                                    for j in range(TH // 512)]
                                   for e in range(4)]
                            for kc in range(KC):
                                wtile = pa.tile([128, F], F32R, tag="w", bufs=4)
                                weng = nc.sync if kc % 2 == 0 else nc.gpsimd
                                weng.dma_start(wtile[:],
                                               wdram[ts(kc, 128), :])
                                for e in range(4):
                                    for j in range(TH // 512):
                                        nc.tensor.matmul(
                                            acc[e][j][:],
                                            r(wtile[:, ts(e, 128)], "qk"),
                                            r(ht[:, kc, ts(j, 512)], "qk"),
                                            start=(kc == 0),
                                            stop=(kc == KC - 1))
                            for e in range(4):
                                for j in range(TH // 512):
                                    csl = slice(half * TH + j * 512,
                                                half * TH + (j + 1) * 512)
                                    ps = acc[e][j]
                                    tmp = pa.tile([128, 512], F32, tag="rt")
                                    nc.vector.tensor_mul(
                                        tmp[0:64, :], ps[64:128, :],
                                        sin_t[0:64, csl])
                                    nc.vector.tensor_mul(
                                        tmp[64:128, :], ps[0:64, :],
                                        sin_t[64:128, csl])
                                    cosp = pa.tile([128, 512], F32, tag="rc")
                                    nc.vector.tensor_mul(cosp[:], ps[:],
                                                         cos_t[:, csl])
                                    nc.vector.tensor_add(dest[e][:, csl],
                                                         tmp[:], cosp[:])

                        # v in natural [token, feature] layout (h^T stationary)
                        if SKIP_V:
                            continue
                        vacc = [psa.tile([128, 512], F32, tag="a",
                                         name=f"va{half}_{u}")
                                for u in range(TH // 128)]
                        for kc in range(KC):
                            wtile = pa.tile([128, F], F32R, tag="w", bufs=4)
                            nc.sync.dma_start(wtile[:], wvt[ts(kc, 128), :])
                            for u in range(TH // 128):
                                nc.tensor.matmul(
                                    vacc[u][:], r(ht[:, kc, ts(u, 128)], "v"),
                                    r(wtile[:], "v"),
                                    start=(kc == 0), stop=(kc == KC - 1))
                        for u in range(TH // 128):
                            nc.vector.tensor_copy(
                                vtiles[half * (TH // 128) + u][:], vacc[u][:])

                # ------------- Phase B: attention + out-proj -------------
                if SKIP_B:
                    return
                with (
                    tc.tile_pool(name="pb", bufs=1) as pb,
                    tc.tile_pool(name="ps_sc", bufs=2, space="PSUM") as ps_sc,
                    tc.tile_pool(name="ps_av", bufs=4, space="PSUM") as ps_av,
                    tc.tile_pool(name="ps_po", bufs=2, space="PSUM") as ps_po,
                ):
                    wo_tiles = [pb.tile([128, D_MODEL], F32R, name=f"wo{f}", tag=f"wo{f}")
                                for f in range(HEADS_PER_CORE)]
                    for f in range(HEADS_PER_CORE):
                        nc.sync.dma_start(wo_tiles[f][:], wot[ts(f, 128), :])

                    for tt in range(T // 512):
                        oT = [pb.tile([128, 512], F32R, tag=f"oT{h}", bufs=2, name=f"oT{h}")
                              for h in range(HEADS_PER_CORE)]
                        for h in range(HEADS_PER_CORE):
                            out_ps = ps_av.tile([128, 512], F32, tag="av")
                            den_ps = ps_av.tile([128, 512], F32, tag="av")
                            for ut in range(KC):
                                sc = ps_sc.tile([128, 512], F32, tag="sc")
                                nc.tensor.matmul(
                                    sc[:], r(krope[h][:, ts(ut, 128)], "sc"),
                                    r(qrope[h][:, ts(tt, 512)], "sc"))
                                at = pb.tile([128, 512], F32R, tag="at", bufs=4)
                                if SKIP_EXP:
                                    nc.vector.tensor_copy(at[:], sc[:])
                                else:
                                    nc.scalar.activation(at[:], sc[:], Act.Exp,
                                                         scale=SCALE)
                                nc.tensor.matmul(
                                    out_ps[:],
                                    r(vtiles[ut][:, ts(h, 128)], "av"),
                                    r(at[:], "av"),
                                    start=(ut == 0), stop=(ut == KC - 1))
                                if not SKIP_DEN:
                                    nc.tensor.matmul(
                                        den_ps[:], r(ones_t[:], "den"),
                                        r(at[:], "den"),
                                        start=(ut == 0), stop=(ut == KC - 1))
                            rec = pb.tile([128, 512], F32, tag="rec", bufs=2)
                            if SKIP_DEN:
                                nc.vector.tensor_copy(oT[h][:], out_ps[:])
                            else:
                                nc.vector.reciprocal(rec[:], den_ps[:])
                                nc.vector.tensor_mul(oT[h][:], out_ps[:],
                                                     rec[:])

                        for et in range(D_MODEL // 128):
                            pps = ps_po.tile([128, 512], F32, tag="po")
                            for fc in range(HEADS_PER_CORE):
                                nc.tensor.matmul(
                                    pps[:],
                                    r(wo_tiles[fc][:, ts(et, 128)], "po"),
                                    r(oT[fc][:], "po"),
                                    start=(fc == 0),
                                    stop=(fc == HEADS_PER_CORE - 1))
                            posb = pb.tile([128, 512], F32, tag="posb", bufs=3)
                            nc.vector.tensor_copy(posb[:], pps[:])
                            nc.sync.dma_start(po[ts(et, 128), ts(tt, 512)],
                                              posb[:])

            if reps == 1:
                emit_body()
            else:
                # hardware loop: NEFF size is independent of `reps`, so
                # wall(reps=R) - wall(reps=1) == (R-1) * body-exec-time
                with tc.For_i(0, reps, 1):
                    emit_body()

    nc.compile()
    _PROGRAMS[key] = nc
    return nc


def _rope_tables():
    inv_freq = 1.0 / (ROPE_BASE ** (np.arange(0, HEAD_DIM, 2,
                                              dtype=np.float32) / HEAD_DIM))
    t = np.arange(T, dtype=np.float32)
    freqs = np.outer(t, inv_freq)                      # (T, 64)
    emb = np.concatenate([freqs, freqs], axis=-1)      # (T, 128)
    cosT = np.ascontiguousarray(np.cos(emb).T).astype(np.float32)
    sinrotT = np.ascontiguousarray(np.sin(emb).T).astype(np.float32)
    sinrotT[0:64, :] *= -1.0
    return cosT, sinrotT


def _shard_inputs(x, norm_w, w_qkv, w_out):
    cosT, sinrotT = _rope_tables()
    nw = norm_w.astype(np.float32)
    in_maps = []
    for c in range(N_CORES):
        b, hg = divmod(c, HG)
        rs = slice(hg * F, (hg + 1) * F)
        ks = slice(D_MODEL + hg * F, D_MODEL + (hg + 1) * F)
        vs = slice(2 * D_MODEL + hg * F, 2 * D_MODEL + (hg + 1) * F)
        in_maps.append({
            "xt": np.ascontiguousarray(x[b].T),
            "wqt": np.ascontiguousarray((w_qkv[rs] * nw).T),
            "wkt": np.ascontiguousarray((w_qkv[ks] * nw).T),
            "wvt": np.ascontiguousarray((w_qkv[vs] * nw).T),
            "wot": np.ascontiguousarray(w_out[:, rs].T),
            "cost": cosT,
            "sint": sinrotT,
            "onesd": np.ones((128, 128), dtype=np.float32),
        })
    return in_maps


def _gather(results, x):
    y = np.empty((B, T, D_MODEL), dtype=np.float32)
    for b in range(B):
        acc = np.asarray(x[b], dtype=np.float32).copy()
        for hg in range(HG):
            acc += results[b * HG + hg]["po"].T
        y[b] = acc
    return y


def run(x, norm_w, w_qkv, w_out, reps: int = 1):
    from concourse.bass_utils import run_bass_kernel_spmd

    nc = _build_program(reps)
    in_maps = _shard_inputs(x, norm_w, w_qkv, w_out)
    res = run_bass_kernel_spmd(nc, in_maps, core_ids=list(range(N_CORES)))
    return _gather(res.results, x)


def kernel(x, norm_w, w_qkv, w_out):
    last_err = None
    for _attempt in range(3):
        try:
            return run(np.asarray(x), np.asarray(norm_w), np.asarray(w_qkv),
                       np.asarray(w_out))
        except Exception as e:  # transient NRT_EXEC_UNIT_UNRECOVERABLE etc.
            last_err = e
    raise last_err
